# revision 6
# baseline (speedup 1.0000x reference)
"""Self-attention (SAGAN-style) Trainium2 kernel, v2.

Reference computation (per batch sample):
    theta = w_theta @ x            # [32, 4096]
    phi   = pool2x2(w_phi @ x)     # [32, 1024]
    g     = pool2x2(w_g @ x)       # [128, 1024]
    beta  = softmax(theta.T @ phi, axis=-1)   # [4096, 1024]
    attn  = g @ beta.T             # [128, 4096]
    out   = gamma * (w_o @ attn) + x

Sharding: data-parallel over batch; B=16 over 8 cores -> 2 samples/core.

v2 strategy (per core, per sample):
  - x loaded ONCE as fp32 via HWDGE (sync queue); projections consume it as
    float32r moving operand (1 cyc/row at >=256 cols), killing the bf16
    casting DMAs and the residual re-read of the baseline.
  - wq packs [th, th, ph, ph]; th/ph evacuated+duplicated so the K=32 score
    matmuls run 4-way row-tiled (tile_position (0,0)/(32,0)/(64,0)/(96,0)).
  - phi/g maxpools fused with PSUM evacuation (DVE reads PSUM strided);
    h-pair pools on GpSimd from SBUF.
  - exp on ScalarE straight out of PSUM -> bf16 SBUF (logits O(+-40), safe).
  - attn + softmax-denominator accumulated per 512-col chunk on the PE
    (g^T stationary / all-ones stationary); normalize via
    reciprocal_approx_fast + STT; out-proj into one 2-bank PSUM tile;
    residual add fused in a single [128,1024] STT against resident fp32 x.
  - output stores on gpsimd SWDGE so sync ring stays dedicated to x loads.
"""

import numpy as np

import concourse.bacc as bacc
import concourse.mybir as mybir
from concourse import tile
from concourse.bass_utils import run_bass_kernel_spmd

F32 = mybir.dt.float32
F32R = mybir.dt.float32r
BF16 = mybir.dt.bfloat16

B, C, H, W = 16, 256, 64, 64
N = H * W            # 4096
M = N // 4           # 1024
C8 = C // 8          # 32
C2 = C // 2          # 128
NCORES = 8
BPC = B // NCORES    # 2 samples per core
NCH = 512            # n-chunk width for matmul streaming
NNCH = N // NCH      # 8
MC = M // 128        # 8 m-chunks


def build_kernel():
    nc = bacc.Bacc("TRN2", target_bir_lowering=False, debug=False)

    x_d = nc.declare_dram_parameter("x", [BPC, C, N], F32R, isOutput=False)
    # [cc][128 chans][th th ph ph] and [cc][128 chans][g]
    wq_d = nc.declare_dram_parameter("wq", [2, 128, 128], F32R, isOutput=False)
    wg_d = nc.declare_dram_parameter("wg", [2, 128, C2], F32R, isOutput=False)
    wo_d = nc.declare_dram_parameter("wo", [C2, C], F32, isOutput=False)  # (gamma*w_o).T
    id_d = nc.declare_dram_parameter("ident", [128, 128], F32, isOutput=False)
    out_d = nc.declare_dram_parameter("out", [BPC, C, N], F32, isOutput=True)

    with tile.TileContext(nc) as tc:
        with (
            tc.tile_pool(name="const", bufs=1) as constp,
            tc.tile_pool(name="xsb", bufs=2) as xp,
            tc.tile_pool(name="proj", bufs=1) as projp,
            tc.tile_pool(name="exp", bufs=1) as expp,
            tc.tile_pool(name="gt", bufs=1) as gtp,
            tc.tile_pool(name="small", bufs=3) as smallp,
            tc.tile_pool(name="outs", bufs=3) as outp,
            tc.tile_pool(name="ps_big", bufs=2, space="PSUM") as psS,
            tc.tile_pool(name="ps_a", bufs=1, space="PSUM") as psA,
            tc.tile_pool(name="ps_d", bufs=1, space="PSUM") as psD,
            tc.tile_pool(name="ps_o", bufs=1, space="PSUM") as psO,
        ):
            # ---- constants / weights ----
            wq, wg = [], []
            for cc in range(2):
                t = constp.tile([128, 128], F32R, tag=f"wq{cc}")
                nc.sync.dma_start(t[:], wq_d[cc])
                wq.append(t)
                t = constp.tile([128, C2], F32R, tag=f"wg{cc}")
                nc.sync.dma_start(t[:], wg_d[cc])
                wg.append(t)
            wo = constp.tile([C2, C], BF16, tag="wo")
            nc.gpsimd.dma_start(wo[:], wo_d[:])  # casting DMA (small)
            id_b = constp.tile([128, 128], BF16, tag="id_b")
            nc.gpsimd.dma_start(id_b[:], id_d[:])
            ones = constp.tile([128, 128], BF16, tag="ones")
            nc.gpsimd.memset(ones[:], 1.0)

            pending = []

            def pop_unit():
                if pending:
                    pending.pop(0)()

            for b in range(BPC):
                # ---- load x fp32 once (HWDGE on sync), chunk-interleaved ----
                x1 = xp.tile([128, 2, N], F32R, tag="x1", name=f"x1_{b}")
                for i in range(4):
                    sl = slice(i * 1024, (i + 1) * 1024)
                    for cc in range(2):
                        nc.sync.dma_start(x1[:, cc, sl],
                                          x_d[b, cc * 128:(cc + 1) * 128, sl])

                # ---- projections (f32r moving), fused pool-evac ----
                # th4: [th th th th] along partitions; ph4: [ph ph ph ph]
                th4 = projp.tile([128, N], BF16, tag="th4", name=f"th4_{b}")
                ph2t = projp.tile([64, N // 2], BF16, tag="ph2t", name=f"ph2t_{b}")
                g2t = projp.tile([C2, N // 2], BF16, tag="g2t", name=f"g2t_{b}")
                ph4 = projp.tile([128, M], BF16, tag="ph4", name=f"ph4_{b}")
                gp = projp.tile([C2, M], BF16, tag="g_p", name=f"gp_{b}")
                for i in range(NNCH):
                    sl = slice(i * NCH, (i + 1) * NCH)
                    csl = slice(i * 256, (i + 1) * 256)
                    msl = slice(i * 128, (i + 1) * 128)
                    ps1 = psS.tile([128, NCH], F32, tag="big", name=f"ps1_{b}_{i}")
                    for cc in range(2):
                        nc.tensor.matmul(ps1[:], wq[cc][:],
                                         x1[:, cc, sl],
                                         start=(cc == 0), stop=(cc == 1))
                    # theta rows 0:64 -> SBUF bf16; phi rows 64:128 w-pooled
                    nc.vector.tensor_copy(th4[0:64, sl], ps1[0:64])
                    pv = ps1[64:128].rearrange("p (w2 two) -> p w2 two", two=2)
                    nc.vector.tensor_reduce(ph2t[:, csl], pv[:],
                                            mybir.AxisListType.X,
                                            mybir.AluOpType.max)
                    ps2 = psS.tile([128, NCH], F32, tag="big", name=f"ps2_{b}_{i}")
                    for cc in range(2):
                        nc.tensor.matmul(ps2[:], wg[cc][:],
                                         x1[:, cc, sl],
                                         start=(cc == 0), stop=(cc == 1))
                    pv2 = ps2[:].rearrange("p (w2 two) -> p w2 two", two=2)
                    nc.vector.tensor_reduce(g2t[:, csl], pv2[:],
                                            mybir.AxisListType.X,
                                            mybir.AluOpType.max)
                    # h-pair pools for this chunk's 128 m-cols
                    vph = ph2t[:, csl].rearrange(
                        "p (h2 hb w2) -> p h2 w2 hb", hb=2, w2=W // 2)
                    nc.vector.tensor_reduce(ph4[0:64, msl], vph[:],
                                            mybir.AxisListType.X,
                                            mybir.AluOpType.max)
                    vg = g2t[:, csl].rearrange(
                        "p (h2 hb w2) -> p h2 w2 hb", hb=2, w2=W // 2)
                    nc.vector.tensor_reduce(gp[:, msl], vg[:],
                                            mybir.AxisListType.X,
                                            mybir.AluOpType.max)
                    pop_unit()

                # duplicate th/ph into upper 64 partitions for 4-way tiling
                nc.vector.tensor_copy(th4[64:128, :], th4[0:64, :])
                nc.vector.tensor_copy(ph4[64:128, :], ph4[0:64, :])

                # gT transposes are emitted inside quarter 0 as PE fillers
                gts = []

                def emit_transpose(mc, b=b):
                    tp = psA.tile([128, 128], BF16, tag="a", name=f"tp{b}_{mc}")
                    nc.tensor.transpose(tp[:], gp[:, mc * 128:(mc + 1) * 128],
                                        id_b[:])
                    gt = gtp.tile([128, 128], BF16, tag=f"gt{mc}", name=f"gt{mc}_{b}")
                    nc.scalar.copy(gt[:], tp[:])
                    gts.append(gt)

                ets = []
                for mc in range(MC):
                    et = expp.tile([128, N], BF16, tag=f"expT{mc}", name=f"expT{mc}_{b}")
                    ets.append(et)

                aps_map = {}

                def unit_attn(i, b=b, ets=ets, gts=gts, aps_map=aps_map, x1=x1):
                    nsl = slice(i * NCH, (i + 1) * NCH)
                    aps = psA.tile([128, NCH], F32, tag="a", name=f"aps{b}_{i}")
                    aps_map[i] = aps
                    for mc in range(MC):
                        nc.tensor.matmul(aps[:], gts[mc][:], ets[mc][:, nsl],
                                         start=(mc == 0), stop=(mc == MC - 1),
                                         skip_group_check=True)

                def unit_den_epi(i, b=b, ets=ets, gts=gts, aps_map=aps_map, x1=x1):
                    nsl = slice(i * NCH, (i + 1) * NCH)
                    aps = aps_map.pop(i)
                    dps = psD.tile([128, NCH], F32, tag="d", name=f"dps{b}_{i}")
                    for mc in range(MC):
                        nc.tensor.matmul(dps[:], ones[:], ets[mc][:, nsl],
                                         start=(mc == 0), stop=(mc == MC - 1),
                                         skip_group_check=True)
                    rec = smallp.tile([128, NCH], F32, tag="rec", name=f"rec{b}_{i}")
                    nc.vector.reciprocal_approx_fast(rec[:], dps[:])
                    at = smallp.tile([128, NCH], BF16, tag="attn", name=f"at{b}_{i}")
                    nc.vector.scalar_tensor_tensor(
                        at[:], aps[:], 1.0, rec[:],
                        mybir.AluOpType.bypass, mybir.AluOpType.mult)
                    pso = psO.tile([128, 2, NCH], F32, tag="o", name=f"pso{b}_{i}")
                    nc.tensor.matmul(pso[:, 0], wo[:, 0:128], at[:],
                                     start=True, stop=True)
                    nc.tensor.matmul(pso[:, 1], wo[:, 128:256], at[:],
                                     start=True, stop=True)
                    osb = outp.tile([128, 2, NCH], F32, tag="osb",
                                    name=f"osb{b}_{i}")
                    nc.vector.scalar_tensor_tensor(
                        osb[:], pso[:], 1.0, x1[:, :, nsl].bitcast(F32),
                        mybir.AluOpType.bypass, mybir.AluOpType.add)
                    # out[b, (cc p), nsl] <- osb[p, cc, :]
                    ov = out_d[b, :, nsl].rearrange("(cc p) n -> p cc n", p=128)
                    nc.gpsimd.dma_start(ov, osb[:])

                for i in range(NNCH):
                    pending.append(lambda f=unit_attn, i=i: f(i))
                    pending.append(lambda f=unit_den_epi, i=i: f(i))

                for qt in range(5):
                    if qt < 4:
                        qsl = slice(qt * 1024, (qt + 1) * 1024)
                        for r in range(4):
                            mc_a, mc_b = 2 * r, 2 * r + 1
                            ca = slice(mc_a * 128, (mc_a + 1) * 128)
                            cb = slice(mc_b * 128, (mc_b + 1) * 128)
                            h0 = slice(qt * 1024, qt * 1024 + 512)
                            h1 = slice(qt * 1024 + 512, (qt + 1) * 1024)
                            spa = psS.tile([128, 1024], F32, tag="big",
                                           name=f"spa{b}_{qt}_{r}")
                            spb = psS.tile([128, 1024], F32, tag="big",
                                           name=f"spb{b}_{qt}_{r}")
                            nc.tensor.matmul(spa[:, 0:512], ph4[0:32, ca],
                                             th4[0:32, h0], start=True, stop=True,
                                             tile_position=(0, 0))
                            nc.tensor.matmul(spb[:, 0:512], ph4[32:64, cb],
                                             th4[32:64, h0], start=True, stop=True,
                                             tile_position=(32, 0))
                            nc.tensor.matmul(spa[:, 512:1024], ph4[64:96, ca],
                                             th4[64:96, h1], start=True, stop=True,
                                             tile_position=(64, 0))
                            nc.tensor.matmul(spb[:, 512:1024], ph4[96:128, cb],
                                             th4[96:128, h1], start=True, stop=True,
                                             tile_position=(96, 0))
                            nc.scalar.activation(ets[mc_a][:, qsl], spa[:],
                                                 mybir.ActivationFunctionType.Exp)
                            nc.scalar.activation(ets[mc_b][:, qsl], spb[:],
                                                 mybir.ActivationFunctionType.Exp)
                            if qt == 0:
                                emit_transpose(2 * r)
                                emit_transpose(2 * r + 1)
                            else:
                                pop_unit()
                    else:
                        keep = 4 if b == 0 else 0
                        while len(pending) > keep:
                            pop_unit()

    nc.compile()
    return nc


_NC_CACHE = None


def _get_nc():
    global _NC_CACHE
    if _NC_CACHE is None:
        _NC_CACHE = build_kernel()
    return _NC_CACHE


def prep_inputs(x, w_theta, w_phi, w_g, w_o, gamma):
    """Host-side prep: shard x over 8 cores; transpose/scale/pack weights."""
    x = np.asarray(x, dtype=np.float32).reshape(B, C, N)
    w_theta = np.asarray(w_theta, dtype=np.float32)
    w_phi = np.asarray(w_phi, dtype=np.float32)
    w_g = np.asarray(w_g, dtype=np.float32)
    w_o = np.asarray(w_o, dtype=np.float32)
    gamma = np.float32(gamma)

    # combined projection weight: [th th ph ph] along output dim
    wqT = np.concatenate([w_theta.T, w_theta.T, w_phi.T, w_phi.T], axis=1)  # [256,128]
    wq = np.ascontiguousarray(wqT.reshape(2, 128, 128))
    wgq = np.ascontiguousarray(w_g.T.reshape(2, 128, C2))
    wo = np.ascontiguousarray((gamma * w_o).T)
    ident = np.eye(128, dtype=np.float32)

    in_maps = []
    for core in range(NCORES):
        shard = np.ascontiguousarray(x[core * BPC:(core + 1) * BPC])
        in_maps.append({"x": shard, "wq": wq, "wg": wgq, "wo": wo, "ident": ident})
    return in_maps


def run(inputs, trace=False, **kw):
    nc = _get_nc()
    in_maps = prep_inputs(**inputs)
    res = run_bass_kernel_spmd(nc, in_maps, core_ids=list(range(NCORES)),
                               trace=trace, **kw)
    outs = [res.results[i]["out"] for i in range(NCORES)]
    full = np.concatenate(outs, axis=0).reshape(B, C, H, W).astype(np.float32)
    return full, res


def kernel(**inputs):
    full, _ = run(inputs, trace=False)
    return full


# revision 7
# speedup vs baseline: 1.0482x; 1.0482x over previous
"""Self-attention (SAGAN-style) Trainium2 kernel, v3.

Reference computation (per batch sample):
    theta = w_theta @ x            # [32, 4096]
    phi   = pool2x2(w_phi @ x)     # [32, 1024]
    g     = pool2x2(w_g @ x)       # [128, 1024]
    beta  = softmax(theta.T @ phi, axis=-1)   # [4096, 1024]
    attn  = g @ beta.T             # [128, 4096]
    out   = gamma * (w_o @ attn) + x

Sharding: data-parallel over batch; B=16 over 8 cores -> 2 samples/core.

v3 strategy (per core, per sample):
  - x loaded as bf16 via casting DMAs (gpsimd SWDGE), also used for the
    residual add (bf16 residual costs ~0.2% rel err, budget is 2e-2).
  - wq packs [th, th, ph, ph]; th evacuated once and duplicated so the K=32
    score matmuls run 4-way row-tiled ((0,0)/(32,0)/(64,0)/(96,0)).
  - 2x2 maxpool of phi/g fused into PSUM evacuation as a single
    tensor_reduce(axis=XY) per chunk per tensor.
  - exp on ScalarE straight out of PSUM -> bf16 SBUF (logits O(+-40), safe).
  - attn + softmax-denominator accumulated per 512-col chunk on the PE
    (g^T stationary / all-ones stationary); normalize via
    reciprocal_approx_fast + STT; out-proj into one 2-bank PSUM tile;
    residual add fused in a single [128,1024] STT against bf16 x.
  - output stores on sync HWDGE; x casting loads on gpsimd SWDGE.
"""

import numpy as np

import concourse.bacc as bacc
import concourse.mybir as mybir
from concourse import tile
from concourse.bass_utils import run_bass_kernel_spmd

F32 = mybir.dt.float32
BF16 = mybir.dt.bfloat16

B, C, H, W = 16, 256, 64, 64
N = H * W            # 4096
M = N // 4           # 1024
C8 = C // 8          # 32
C2 = C // 2          # 128
NCORES = 8
BPC = B // NCORES    # 2 samples per core
NCH = 512            # n-chunk width for matmul streaming
NNCH = N // NCH      # 8
MC = M // 128        # 8 m-chunks


def build_kernel():
    nc = bacc.Bacc("TRN2", target_bir_lowering=False, debug=False)

    x_d = nc.declare_dram_parameter("x", [BPC, C, N], F32, isOutput=False)
    # [cc][128 chans][th th ph ph] and [cc][128 chans][g]
    wq_d = nc.declare_dram_parameter("wq", [2, 128, 128], F32, isOutput=False)
    wg_d = nc.declare_dram_parameter("wg", [2, 128, C2], F32, isOutput=False)
    wo_d = nc.declare_dram_parameter("wo", [C2, C], F32, isOutput=False)  # (gamma*w_o).T
    id_d = nc.declare_dram_parameter("ident", [128, 128], F32, isOutput=False)
    out_d = nc.declare_dram_parameter("out", [BPC, C, N], F32, isOutput=True)

    with tile.TileContext(nc) as tc:
        with (
            tc.tile_pool(name="const", bufs=1) as constp,
            tc.tile_pool(name="xsb", bufs=2) as xp,
            tc.tile_pool(name="proj", bufs=1) as projp,
            tc.tile_pool(name="exp", bufs=1) as expp,
            tc.tile_pool(name="gt", bufs=1) as gtp,
            tc.tile_pool(name="small", bufs=3) as smallp,
            tc.tile_pool(name="outs", bufs=3) as outp,
            tc.tile_pool(name="ps_big", bufs=2, space="PSUM") as psS,
            tc.tile_pool(name="ps_a", bufs=1, space="PSUM") as psA,
            tc.tile_pool(name="ps_d", bufs=1, space="PSUM") as psD,
            tc.tile_pool(name="ps_o", bufs=1, space="PSUM") as psO,
        ):
            # ---- constants / weights (casting DMAs, small) ----
            wq, wg = [], []
            for cc in range(2):
                t = constp.tile([128, 128], BF16, tag=f"wq{cc}")
                nc.gpsimd.dma_start(t[:], wq_d[cc])
                wq.append(t)
                t = constp.tile([128, C2], BF16, tag=f"wg{cc}")
                nc.gpsimd.dma_start(t[:], wg_d[cc])
                wg.append(t)
            wo = constp.tile([C2, C], BF16, tag="wo")
            nc.gpsimd.dma_start(wo[:], wo_d[:])
            id_b = constp.tile([128, 128], BF16, tag="id_b")
            nc.gpsimd.dma_start(id_b[:], id_d[:])
            ones = constp.tile([128, 128], BF16, tag="ones")
            nc.gpsimd.memset(ones[:], 1.0)

            pending = []

            def pop_unit():
                if pending:
                    pending.pop(0)()

            for b in range(BPC):
                # ---- load x as bf16 (casting DMAs on gpsimd SWDGE) ----
                xbf = xp.tile([128, 2, N], BF16, tag="xbf", name=f"xbf_{b}")
                xdv = x_d[b].rearrange("(cc p) n -> p cc n", p=128)
                for i in range(4):
                    sl = slice(i * 1024, (i + 1) * 1024)
                    nc.gpsimd.dma_start(xbf[:, :, sl], xdv[:, :, sl])

                # ---- projections, fused pool-evac ----
                # th4: [th th th th] along partitions; ph4: [ph ph ph ph]
                th4 = projp.tile([128, N], BF16, tag="th4", name=f"th4_{b}")
                ph4 = projp.tile([128, M], BF16, tag="ph4", name=f"ph4_{b}")
                gp = projp.tile([C2, M], BF16, tag="g_p", name=f"gp_{b}")
                for i in range(NNCH):
                    sl = slice(i * NCH, (i + 1) * NCH)
                    msl = slice(i * 128, (i + 1) * 128)
                    ps1 = psS.tile([128, NCH], F32, tag="big", name=f"ps1_{b}_{i}")
                    for cc in range(2):
                        nc.tensor.matmul(ps1[:], wq[cc][:], xbf[:, cc, sl],
                                         start=(cc == 0), stop=(cc == 1))
                    # theta rows 0:64 -> SBUF bf16; phi rows 64:128 pooled 2x2
                    nc.vector.tensor_copy(th4[0:64, sl], ps1[0:64])
                    pv = ps1[64:128].rearrange(
                        "p (h2 hb w2 wb) -> p h2 w2 hb wb", h2=4, hb=2, wb=2)
                    nc.vector.tensor_reduce(ph4[0:64, msl], pv[:],
                                            mybir.AxisListType.XY,
                                            mybir.AluOpType.max)
                    ps2 = psS.tile([128, NCH], F32, tag="big", name=f"ps2_{b}_{i}")
                    for cc in range(2):
                        nc.tensor.matmul(ps2[:], wg[cc][:], xbf[:, cc, sl],
                                         start=(cc == 0), stop=(cc == 1))
                    pv2 = ps2[:].rearrange(
                        "p (h2 hb w2 wb) -> p h2 w2 hb wb", h2=4, hb=2, wb=2)
                    nc.vector.tensor_reduce(gp[:, msl], pv2[:],
                                            mybir.AxisListType.XY,
                                            mybir.AluOpType.max)
                    pop_unit()

                # duplicate th/ph into upper 64 partitions for 4-way tiling
                nc.vector.tensor_copy(th4[64:128, :], th4[0:64, :])
                nc.vector.tensor_copy(ph4[64:128, :], ph4[0:64, :])

                # gT transposes are emitted inside quarter 0 as PE fillers
                gts = []

                def emit_transpose(mc, b=b):
                    tp = psA.tile([128, 128], BF16, tag="a", name=f"tp{b}_{mc}")
                    nc.tensor.transpose(tp[:], gp[:, mc * 128:(mc + 1) * 128],
                                        id_b[:])
                    gt = gtp.tile([128, 128], BF16, tag=f"gt{mc}", name=f"gt{mc}_{b}")
                    nc.vector.tensor_copy(gt[:], tp[:])
                    gts.append(gt)

                ets = []
                for mc in range(MC):
                    et = expp.tile([128, N], BF16, tag=f"expT{mc}", name=f"expT{mc}_{b}")
                    ets.append(et)

                aps_map = {}

                def unit_attn(i, b=b, ets=ets, gts=gts, aps_map=aps_map, xbf=xbf):
                    nsl = slice(i * NCH, (i + 1) * NCH)
                    aps = psA.tile([128, NCH], F32, tag="a", name=f"aps{b}_{i}")
                    aps_map[i] = aps
                    for mc in range(MC):
                        nc.tensor.matmul(aps[:], gts[mc][:], ets[mc][:, nsl],
                                         start=(mc == 0), stop=(mc == MC - 1),
                                         skip_group_check=True)

                def unit_den_epi(i, b=b, ets=ets, gts=gts, aps_map=aps_map, xbf=xbf):
                    nsl = slice(i * NCH, (i + 1) * NCH)
                    aps = aps_map.pop(i)
                    dps = psD.tile([128, NCH], F32, tag="d", name=f"dps{b}_{i}")
                    for mc in range(MC):
                        nc.tensor.matmul(dps[:], ones[:], ets[mc][:, nsl],
                                         start=(mc == 0), stop=(mc == MC - 1),
                                         skip_group_check=True)
                    rec = smallp.tile([128, NCH], F32, tag="rec", name=f"rec{b}_{i}")
                    nc.vector.reciprocal_approx_fast(rec[:], dps[:])
                    at = smallp.tile([128, NCH], BF16, tag="attn", name=f"at{b}_{i}")
                    nc.vector.scalar_tensor_tensor(
                        at[:], aps[:], 1.0, rec[:],
                        mybir.AluOpType.bypass, mybir.AluOpType.mult)
                    pso = psO.tile([128, 2, NCH], F32, tag="o", name=f"pso{b}_{i}")
                    nc.tensor.matmul(pso[:, 0], wo[:, 0:128], at[:],
                                     start=True, stop=True)
                    nc.tensor.matmul(pso[:, 1], wo[:, 128:256], at[:],
                                     start=True, stop=True)
                    osb = outp.tile([128, 2, NCH], F32, tag="osb",
                                    name=f"osb{b}_{i}")
                    nc.vector.scalar_tensor_tensor(
                        osb[:], pso[:], 1.0, xbf[:, :, nsl],
                        mybir.AluOpType.bypass, mybir.AluOpType.add)
                    # out[b, (cc p), nsl] <- osb[p, cc, :]
                    ov = out_d[b, :, nsl].rearrange("(cc p) n -> p cc n", p=128)
                    nc.sync.dma_start(ov, osb[:])

                for i in range(NNCH):
                    pending.append(lambda f=unit_attn, i=i: f(i))
                    pending.append(lambda f=unit_den_epi, i=i: f(i))

                for qt in range(5):
                    if qt < 4:
                        qsl = slice(qt * 1024, (qt + 1) * 1024)
                        for r in range(4):
                            mc_a, mc_b = 2 * r, 2 * r + 1
                            ca = slice(mc_a * 128, (mc_a + 1) * 128)
                            cb = slice(mc_b * 128, (mc_b + 1) * 128)
                            h0 = slice(qt * 1024, qt * 1024 + 512)
                            h1 = slice(qt * 1024 + 512, (qt + 1) * 1024)
                            spa = psS.tile([128, 1024], F32, tag="big",
                                           name=f"spa{b}_{qt}_{r}")
                            spb = psS.tile([128, 1024], F32, tag="big",
                                           name=f"spb{b}_{qt}_{r}")
                            nc.tensor.matmul(spa[:, 0:512], ph4[0:32, ca],
                                             th4[0:32, h0], start=True, stop=True,
                                             tile_position=(0, 0))
                            nc.tensor.matmul(spb[:, 0:512], ph4[32:64, cb],
                                             th4[32:64, h0], start=True, stop=True,
                                             tile_position=(32, 0))
                            nc.tensor.matmul(spa[:, 512:1024], ph4[64:96, ca],
                                             th4[64:96, h1], start=True, stop=True,
                                             tile_position=(64, 0))
                            nc.tensor.matmul(spb[:, 512:1024], ph4[96:128, cb],
                                             th4[96:128, h1], start=True, stop=True,
                                             tile_position=(96, 0))
                            nc.scalar.activation(ets[mc_a][:, qsl], spa[:],
                                                 mybir.ActivationFunctionType.Exp)
                            nc.scalar.activation(ets[mc_b][:, qsl], spb[:],
                                                 mybir.ActivationFunctionType.Exp)
                            if qt == 0:
                                emit_transpose(2 * r)
                                emit_transpose(2 * r + 1)
                            else:
                                pop_unit()
                    else:
                        keep = 6 if b == 0 else 0
                        while len(pending) > keep:
                            pop_unit()

    nc.compile()
    return nc


_NC_CACHE = None


def _get_nc():
    global _NC_CACHE
    if _NC_CACHE is None:
        _NC_CACHE = build_kernel()
    return _NC_CACHE


def prep_inputs(x, w_theta, w_phi, w_g, w_o, gamma):
    """Host-side prep: shard x over 8 cores; transpose/scale/pack weights."""
    x = np.asarray(x, dtype=np.float32).reshape(B, C, N)
    w_theta = np.asarray(w_theta, dtype=np.float32)
    w_phi = np.asarray(w_phi, dtype=np.float32)
    w_g = np.asarray(w_g, dtype=np.float32)
    w_o = np.asarray(w_o, dtype=np.float32)
    gamma = np.float32(gamma)

    # combined projection weight: [th th ph ph] along output dim
    wqT = np.concatenate([w_theta.T, w_theta.T, w_phi.T, w_phi.T], axis=1)  # [256,128]
    wq = np.ascontiguousarray(wqT.reshape(2, 128, 128))
    wgq = np.ascontiguousarray(w_g.T.reshape(2, 128, C2))
    wo = np.ascontiguousarray((gamma * w_o).T)
    ident = np.eye(128, dtype=np.float32)

    in_maps = []
    for core in range(NCORES):
        shard = np.ascontiguousarray(x[core * BPC:(core + 1) * BPC])
        in_maps.append({"x": shard, "wq": wq, "wg": wgq, "wo": wo, "ident": ident})
    return in_maps


def run(inputs, trace=False, **kw):
    nc = _get_nc()
    in_maps = prep_inputs(**inputs)
    res = run_bass_kernel_spmd(nc, in_maps, core_ids=list(range(NCORES)),
                               trace=trace, **kw)
    outs = [res.results[i]["out"] for i in range(NCORES)]
    full = np.concatenate(outs, axis=0).reshape(B, C, H, W).astype(np.float32)
    return full, res


def kernel(**inputs):
    full, _ = run(inputs, trace=False)
    return full


# revision 9
# speedup vs baseline: 1.1137x; 1.0625x over previous
"""Self-attention (SAGAN-style) Trainium2 kernel, v4.

Reference computation (per batch sample):
    theta = w_theta @ x            # [32, 4096]
    phi   = pool2x2(w_phi @ x)     # [32, 1024]
    g     = pool2x2(w_g @ x)       # [128, 1024]
    beta  = softmax(theta.T @ phi, axis=-1)   # [4096, 1024]
    attn  = g @ beta.T             # [128, 4096]
    out   = gamma * (w_o @ attn) + x

Sharding: data-parallel over batch; B=16 over 8 cores -> 2 samples/core.

v4 schedule (per core, per sample), from v3 trace evidence:
  - x loaded bf16 via casting DMAs (gpsimd SWDGE); bf16 residual.
  - qt-0 score rounds are INTERLEAVED into the projection loop (round r
    only needs pooled-phi m-chunks 2r/2r+1 = proj chunks 2r/2r+1), so the
    ScalarE exp pipeline primes ~8us earlier per sample and the proj->qt
    phase boundary disappears. g^T transposes also emit per-chunk.
  - th evac on ScalarE (DVE was the proj-phase pacer in v3); 2x2 pools as
    single tensor_reduce(XY) PSUM->SBUF on DVE; th/ph 4-way duplicates
    copied per-chunk.
  - attn+den fused into one 16-matmul PE burst per n-chunk (one exposed
    drain latency); the out-projection unit is popped ~1.5 units later so
    the PE never waits on the reciprocal/normalize DVE chain.
  - 4-way row-tiled score matmuls ((0,0)/(32,0)/(64,0)/(96,0)).
  - output stores on sync HWDGE; gts double-buffered across samples.
"""

import numpy as np

import concourse.bacc as bacc
import concourse.mybir as mybir
from concourse import tile
from concourse.bass_utils import run_bass_kernel_spmd

F32 = mybir.dt.float32
BF16 = mybir.dt.bfloat16

B, C, H, W = 16, 256, 64, 64
N = H * W            # 4096
M = N // 4           # 1024
C8 = C // 8          # 32
C2 = C // 2          # 128
NCORES = 8
BPC = B // NCORES    # 2 samples per core
NCH = 512            # n-chunk width for matmul streaming
NNCH = N // NCH      # 8
MC = M // 128        # 8 m-chunks


def build_kernel():
    nc = bacc.Bacc("TRN2", target_bir_lowering=False, debug=False)

    x_d = nc.declare_dram_parameter("x", [BPC, C, N], F32, isOutput=False)
    # [cc][128 chans][th th ph ph] and [cc][128 chans][g]
    wq_d = nc.declare_dram_parameter("wq", [2, 128, 128], F32, isOutput=False)
    wg_d = nc.declare_dram_parameter("wg", [2, 128, C2], F32, isOutput=False)
    wo_d = nc.declare_dram_parameter("wo", [C2, C], F32, isOutput=False)  # (gamma*w_o).T
    id_d = nc.declare_dram_parameter("ident", [128, 128], F32, isOutput=False)
    out_d = nc.declare_dram_parameter("out", [BPC, C, N], F32, isOutput=True)

    with tile.TileContext(nc) as tc:
        with (
            tc.tile_pool(name="const", bufs=1) as constp,
            tc.tile_pool(name="xsb", bufs=2) as xp,
            tc.tile_pool(name="proj", bufs=1) as projp,
            tc.tile_pool(name="exp", bufs=1) as expp,
            tc.tile_pool(name="gt", bufs=2) as gtp,
            tc.tile_pool(name="small", bufs=3) as smallp,
            tc.tile_pool(name="outs", bufs=3) as outp,
            tc.tile_pool(name="ps_big", bufs=2, space="PSUM") as psS,
            tc.tile_pool(name="ps_a", bufs=1, space="PSUM") as psA,
            tc.tile_pool(name="ps_d", bufs=1, space="PSUM") as psD,
            tc.tile_pool(name="ps_o", bufs=1, space="PSUM") as psO,
        ):
            # ---- constants / weights (casting DMAs, small) ----
            wq, wg = [], []
            for cc in range(2):
                t = constp.tile([128, 128], BF16, tag=f"wq{cc}")
                nc.gpsimd.dma_start(t[:], wq_d[cc])
                wq.append(t)
                t = constp.tile([128, C2], BF16, tag=f"wg{cc}")
                nc.gpsimd.dma_start(t[:], wg_d[cc])
                wg.append(t)
            wo = constp.tile([C2, C], BF16, tag="wo")
            nc.gpsimd.dma_start(wo[:], wo_d[:])
            id_b = constp.tile([128, 128], BF16, tag="id_b")
            nc.gpsimd.dma_start(id_b[:], id_d[:])
            ones = constp.tile([128, 128], BF16, tag="ones")
            nc.gpsimd.memset(ones[:], 1.0)

            pending = []

            def pop_unit():
                if pending:
                    pending.pop(0)()

            for b in range(BPC):
                # ---- load x as bf16 (casting DMAs on gpsimd SWDGE) ----
                xbf = xp.tile([128, 2, N], BF16, tag="xbf", name=f"xbf_{b}")
                xdv = x_d[b].rearrange("(cc p) n -> p cc n", p=128)
                for sl in (slice(0, 512), slice(512, 1536), slice(1536, 2816),
                           slice(2816, 4096)):
                    nc.gpsimd.dma_start(xbf[:, :, sl], xdv[:, :, sl])

                th4 = projp.tile([128, N], BF16, tag="th4", name=f"th4_{b}")
                ph4 = projp.tile([128, M], BF16, tag="ph4", name=f"ph4_{b}")
                gp = projp.tile([C2, M], BF16, tag="g_p", name=f"gp_{b}")
                gts = []
                ets = []
                for mc in range(MC):
                    et = expp.tile([128, N], BF16, tag=f"expT{mc}",
                                   name=f"expT{mc}_{b}")
                    ets.append(et)

                aps_map = {}

                def unit_ad(i, b=b, ets=ets, gts=gts, aps_map=aps_map):
                    """attn + den accumulation: one 16-matmul PE burst, then
                    the reciprocal/normalize DVE chain."""
                    nsl = slice(i * NCH, (i + 1) * NCH)
                    aps = psA.tile([128, NCH], F32, tag="a", name=f"aps{b}_{i}")
                    dps = psD.tile([128, NCH], F32, tag="d", name=f"dps{b}_{i}")
                    for mc in range(MC):
                        nc.tensor.matmul(aps[:], gts[mc][:], ets[mc][:, nsl],
                                         start=(mc == 0), stop=(mc == MC - 1),
                                         skip_group_check=True)
                    for mc in range(MC):
                        nc.tensor.matmul(dps[:], ones[:], ets[mc][:, nsl],
                                         start=(mc == 0), stop=(mc == MC - 1),
                                         skip_group_check=True)
                    rec = smallp.tile([128, NCH], F32, tag="rec", name=f"rec{b}_{i}")
                    nc.vector.reciprocal_approx_fast(rec[:], dps[:])
                    at = smallp.tile([128, NCH], BF16, tag="attn", name=f"at{b}_{i}")
                    nc.vector.scalar_tensor_tensor(
                        at[:], aps[:], 1.0, rec[:],
                        mybir.AluOpType.bypass, mybir.AluOpType.mult)
                    aps_map[i] = at

                def unit_out(i, b=b, aps_map=aps_map, xbf=xbf):
                    nsl = slice(i * NCH, (i + 1) * NCH)
                    at = aps_map.pop(i)
                    pso = psO.tile([128, 2, NCH], F32, tag="o", name=f"pso{b}_{i}")
                    nc.tensor.matmul(pso[:, 0], wo[:, 0:128], at[:],
                                     start=True, stop=True)
                    nc.tensor.matmul(pso[:, 1], wo[:, 128:256], at[:],
                                     start=True, stop=True)
                    osb = outp.tile([128, 2, NCH], F32, tag="osb",
                                    name=f"osb{b}_{i}")
                    nc.vector.scalar_tensor_tensor(
                        osb[:], pso[:], 1.0, xbf[:, :, nsl],
                        mybir.AluOpType.bypass, mybir.AluOpType.add)
                    # out[b, (cc p), nsl] <- osb[p, cc, :]
                    ov = out_d[b, :, nsl].rearrange("(cc p) n -> p cc n", p=128)
                    nc.sync.dma_start(ov, osb[:])

                def emit_round(qt, r, b=b, ets=ets, th4=th4, ph4=ph4):
                    mc_a, mc_b = 2 * r, 2 * r + 1
                    ca = slice(mc_a * 128, (mc_a + 1) * 128)
                    cb = slice(mc_b * 128, (mc_b + 1) * 128)
                    qsl = slice(qt * 1024, (qt + 1) * 1024)
                    h0 = slice(qt * 1024, qt * 1024 + 512)
                    h1 = slice(qt * 1024 + 512, (qt + 1) * 1024)
                    spa = psS.tile([128, 1024], F32, tag="big",
                                   name=f"spa{b}_{qt}_{r}")
                    spb = psS.tile([128, 1024], F32, tag="big",
                                   name=f"spb{b}_{qt}_{r}")
                    nc.tensor.matmul(spa[:, 0:512], ph4[0:32, ca],
                                     th4[0:32, h0], start=True, stop=True,
                                     tile_position=(0, 0))
                    nc.tensor.matmul(spb[:, 0:512], ph4[32:64, cb],
                                     th4[32:64, h0], start=True, stop=True,
                                     tile_position=(32, 0))
                    nc.tensor.matmul(spa[:, 512:1024], ph4[64:96, ca],
                                     th4[64:96, h1], start=True, stop=True,
                                     tile_position=(64, 0))
                    nc.tensor.matmul(spb[:, 512:1024], ph4[96:128, cb],
                                     th4[96:128, h1], start=True, stop=True,
                                     tile_position=(96, 0))
                    nc.scalar.activation(ets[mc_a][:, qsl], spa[:],
                                         mybir.ActivationFunctionType.Exp)
                    nc.scalar.activation(ets[mc_b][:, qsl], spb[:],
                                         mybir.ActivationFunctionType.Exp)

                # ---- projection loop with interleaved qt0 rounds ----
                for i in range(NNCH):
                    sl = slice(i * NCH, (i + 1) * NCH)
                    msl = slice(i * 128, (i + 1) * 128)
                    ps1 = psS.tile([128, NCH], F32, tag="big", name=f"ps1_{b}_{i}")
                    for cc in range(2):
                        nc.tensor.matmul(ps1[:], wq[cc][:], xbf[:, cc, sl],
                                         start=(cc == 0), stop=(cc == 1))
                    # theta rows 0:64 -> SBUF bf16 (ScalarE); phi 2x2-pooled
                    nc.scalar.copy(th4[0:64, sl], ps1[0:64])
                    pv = ps1[64:128].rearrange(
                        "p (h2 hb w2 wb) -> p h2 w2 hb wb", h2=4, hb=2, wb=2)
                    nc.vector.tensor_reduce(ph4[0:64, msl], pv[:],
                                            mybir.AxisListType.XY,
                                            mybir.AluOpType.max)
                    ps2 = psS.tile([128, NCH], F32, tag="big", name=f"ps2_{b}_{i}")
                    for cc in range(2):
                        nc.tensor.matmul(ps2[:], wg[cc][:], xbf[:, cc, sl],
                                         start=(cc == 0), stop=(cc == 1))
                    pv2 = ps2[:].rearrange(
                        "p (h2 hb w2 wb) -> p h2 w2 hb wb", h2=4, hb=2, wb=2)
                    nc.vector.tensor_reduce(gp[:, msl], pv2[:],
                                            mybir.AxisListType.XY,
                                            mybir.AluOpType.max)
                    # 4-way duplicates for this chunk
                    nc.vector.tensor_copy(th4[64:128, sl], th4[0:64, sl])
                    nc.vector.tensor_copy(ph4[64:128, msl], ph4[0:64, msl])
                    # gT transpose for this chunk's m-block
                    tp = psA.tile([128, 128], BF16, tag="a", name=f"tp{b}_{i}")
                    nc.tensor.transpose(tp[:], gp[:, msl], id_b[:])
                    gt = gtp.tile([128, 128], BF16, tag=f"gt{i}", name=f"gt{i}_{b}")
                    nc.vector.tensor_copy(gt[:], tp[:])
                    gts.append(gt)
                    pop_unit()
                    if i % 2 == 1:
                        emit_round(0, (i - 1) // 2)

                # queue this sample's work units: attn+den fused, out lagged
                for i in range(NNCH):
                    pending.append(lambda f=unit_ad, i=i: f(i))
                    if i >= 1:
                        pending.append(lambda f=unit_out, i=i - 1: f(i))
                pending.append(lambda f=unit_out, i=NNCH - 1: f(i))

                # ---- remaining quarters; pop one unit per round ----
                # (leaves 4 units pending to fill the next sample's proj phase)
                for qt in range(1, 4):
                    for r in range(4):
                        emit_round(qt, r)
                        pop_unit()

                if b == BPC - 1:
                    while pending:
                        pop_unit()

    nc.compile()
    return nc


_NC_CACHE = None


def _get_nc():
    global _NC_CACHE
    if _NC_CACHE is None:
        _NC_CACHE = build_kernel()
    return _NC_CACHE


def prep_inputs(x, w_theta, w_phi, w_g, w_o, gamma):
    """Host-side prep: shard x over 8 cores; transpose/scale/pack weights."""
    x = np.asarray(x, dtype=np.float32).reshape(B, C, N)
    w_theta = np.asarray(w_theta, dtype=np.float32)
    w_phi = np.asarray(w_phi, dtype=np.float32)
    w_g = np.asarray(w_g, dtype=np.float32)
    w_o = np.asarray(w_o, dtype=np.float32)
    gamma = np.float32(gamma)

    # combined projection weight: [th th ph ph] along output dim
    wqT = np.concatenate([w_theta.T, w_theta.T, w_phi.T, w_phi.T], axis=1)  # [256,128]
    wq = np.ascontiguousarray(wqT.reshape(2, 128, 128))
    wgq = np.ascontiguousarray(w_g.T.reshape(2, 128, C2))
    wo = np.ascontiguousarray((gamma * w_o).T)
    ident = np.eye(128, dtype=np.float32)

    in_maps = []
    for core in range(NCORES):
        shard = np.ascontiguousarray(x[core * BPC:(core + 1) * BPC])
        in_maps.append({"x": shard, "wq": wq, "wg": wgq, "wo": wo, "ident": ident})
    return in_maps


def run(inputs, trace=False, **kw):
    nc = _get_nc()
    in_maps = prep_inputs(**inputs)
    res = run_bass_kernel_spmd(nc, in_maps, core_ids=list(range(NCORES)),
                               trace=trace, **kw)
    outs = [res.results[i]["out"] for i in range(NCORES)]
    full = np.concatenate(outs, axis=0).reshape(B, C, H, W).astype(np.float32)
    return full, res


def kernel(**inputs):
    full, _ = run(inputs, trace=False)
    return full
